# revision 21
# baseline (speedup 1.0000x reference)
"""Depthwise causal conv1d (W=8) with 3 interleaved weight sets, on 8 TRN2 cores.

Reference computes r/o/a = depthwise_causal_conv(x, {rtg,obs,act}_{w,b}) and
interleaves out[:, t] = {r,o,a}[:, t] by t % 3.  Only the t%3-matching third of
each conv is needed, so total work is exactly one conv: for each output t,
out[b,t,h] = sum_k x[b, t-7+k, h] * w_{t%3}[h, k] + b_{t%3}[h].

Strategy (pure batch data-parallel, B=16 -> 2 per core):
  - host pre-transposes x to channels-major fp16 and PHASE-SPLITS time by t%3
    (x_phase[p][c, u] = x[c, 3u+p], left-padded 3 zeros) so every on-chip
    access is unit-stride
  - on-chip: per (batch, channel-group): the conv runs on the TensorEngine as
    8 accumulating fp16 matmuls per 512-wide output block: lhsT = diag(w_s[:,k])
    (128x128), rhs = the phase-(s+k-7 mod 3) slice of x at unit stride,
    accumulated fp32 in PSUM; 4 consecutive matmuls (2 batches x 2 blocks)
    share each diag and redundant LDWEIGHTS are deduped post-compile
  - ScalarE evicts PSUM with fused per-channel f32 bias into a phase-split
    fp16 output tile (unit-stride)
  - host re-interleaves phases / transposes back / upcasts to f32.
fp16 end-to-end rel err ~6e-4 (x, w quantization + fp16 output rounding).
"""

import os
import numpy as np

B, T, H, W = 16, 3072, 768, 8
NCORES = 8
B_LOC = B // NCORES          # 2 batches per core
G = H // 128                 # 6 channel groups
U = T // 3                   # 1024 per phase
PAD = 3                      # left zero-pad per phase (covers q in {-3..0})
UP = U + PAD                 # 1027 stored per phase
NFREE = 512                  # psum tile width (one fp32 bank)
NT = U // NFREE              # 2 psum tiles per phase

_cache = {}


def _dedupe_ldweights(nc):
    """bacc lowers every 16-bit matmul to an InstLdweights + InstMatmult pair.
    The PE serializes each load (~130ns) before its matmul.  Our loop order
    makes 4 consecutive matmuls share the same diag lhsT, so drop the
    redundant reloads: remove an InstLdweights whose weights AP equals the
    previous one on the PE stream, carrying its semaphore waits onto the next
    PE instruction (reverting bacc's move_matmul_waits_to_ldweights motion).
    The 64B ISA word has one wait slot, so only dedupe when the waits fit."""
    import concourse.mybir as mybir

    removed = 0
    for fn in nc.m.functions:
        for blk in fn.blocks:
            insts = list(blk.instructions)
            drop = set()
            last_key = None
            for i, inst in enumerate(insts):
                if getattr(inst, "engine", None) != mybir.EngineType.PE:
                    continue
                tn = type(inst).__name__
                if tn == "InstLdweights":
                    a = inst.ins[0]
                    key = (a.memref, a.offset, str(a.ap), str(a.dtype))
                    si = inst.sync_info
                    my_waits = list(si.on_wait) if si is not None else []
                    has_upd = si is not None and len(si.on_update) > 0
                    if key == last_key and not has_upd:
                        nxt = None
                        for j in range(i + 1, len(insts)):
                            if getattr(insts[j], "engine", None) == mybir.EngineType.PE:
                                nxt = insts[j]
                                break
                        if nxt is not None:
                            nsi = nxt.sync_info
                            n_waits = len(nsi.on_wait) if nsi is not None else 0
                            if n_waits + len(my_waits) <= 1:
                                if my_waits:
                                    if nsi is None:
                                        nxt.sync_info = mybir.SyncInfo(
                                            on_wait=my_waits, on_update=[]
                                        )
                                    else:
                                        nsi.on_wait = list(nsi.on_wait) + my_waits
                                drop.add(i)
                                removed += 1
                                continue
                    last_key = key
                elif tn == "InstMatmult":
                    pass  # non-self-loading; PE array state unchanged
                else:
                    last_key = None  # be conservative about other PE ops
            if drop:
                blk.instructions = [x for i, x in enumerate(insts) if i not in drop]
    return removed


def _build_nc():
    import concourse.bacc as bacc
    import concourse.mybir as mybir
    import concourse.tile as tile

    nc = bacc.Bacc("TRN2", target_bir_lowering=False, debug=False)
    f32 = mybir.dt.float32
    f16 = mybir.dt.float16

    x_d = nc.dram_tensor("x", [B_LOC, G, 128, 3 * UP], f16, kind="ExternalInput").ap()
    wid_d = nc.dram_tensor("wid", [128, 128], f16, kind="ExternalInput").ap()
    w_d = nc.dram_tensor("w", [128, G * 3 * W], f32, kind="ExternalInput").ap()
    b_d = nc.dram_tensor("b", [128, G * 3], f32, kind="ExternalInput").ap()
    y_d = nc.dram_tensor("y", [B_LOC, G, 128, 3 * U], f16, kind="ExternalOutput").ap()

    with tile.TileContext(nc) as tc:
        with (
            tc.tile_pool(name="const", bufs=1) as constp,
            tc.tile_pool(name="diag", bufs=2) as diagp,
            tc.tile_pool(name="xp", bufs=2) as xp,
            tc.tile_pool(name="op", bufs=2) as op,
            tc.tile_pool(name="dv", bufs=2) as dv,
            tc.tile_pool(name="ps", bufs=2, space="PSUM") as psp,
        ):
            wid = constp.tile([128, 128], f16)
            wt = constp.tile([128, G * 3 * W], f32)
            bt = constp.tile([128, G * 3], f32)
            nc.sync.dma_start(wid[:], wid_d[:])
            nc.sync.dma_start(wt[:], w_d[:])
            nc.sync.dma_start(bt[:], b_d[:])

            for g in range(G):
                # the 24 diagonal fp16 weight matrices for this channel group
                diags = []
                for s in range(3):
                    for k in range(W):
                        c = (g * 3 + s) * W + k
                        dt_ = diagp.tile([128, 128], f16, tag=f"diag{s}_{k}")
                        nc.vector.tensor_scalar_mul(dt_[:], wid[:], wt[:, c : c + 1])
                        diags.append(dt_)
                xts, ots = [], []
                for b in range(B_LOC):
                    xt = xp.tile([128, 3 * UP], f16, tag=f"xt{b}")
                    nc.sync.dma_start(xt[:], x_d[b, g])
                    xts.append(xt)
                    ot = op.tile([128, 3 * U], f16, tag=f"ot{b}")
                    ots.append(ot)
                # Most s=2 blocks run off the TensorEngine to balance the
                # engines: "dve" blocks do tensor_scalar muls + an add tree
                # on VectorE; "coop" blocks do the muls on ScalarE
                # (activation scale=w) and only the add tree on VectorE.
                # Split tuned from the trace: PE 51 / dve 8 / coop 13 blocks.
                def block_role(b, s, nt):
                    if s != 2:
                        return "pe"
                    if b == 1 and nt == 0:
                        return "dve"
                    if b == 1:  # nt == 1
                        return "dve" if g in (0, 1) else "coop"
                    if nt == 0:  # b == 0
                        return "pe" if g in (0, 1, 2) else "coop"
                    return "coop"

                for s in range(3):
                    pe_blocks = [
                        (b, nt)
                        for b in range(B_LOC)
                        for nt in range(NT)
                        if block_role(b, s, nt) == "pe"
                    ]
                    pss = {}
                    for b, nt in pe_blocks:
                        ps = psp.tile([128, NFREE], f32, tag=f"ps{b}_{nt}")
                        pss[b, nt] = ps
                    # k outer: the (b, nt) matmuls of one tap share lhsT,
                    # so the deduper elides the repeated weight loads
                    for k in range(W):
                        o = s + k - (W - 1)          # tap offset in time
                        p, q = o % 3, o // 3         # phase, shift within phase
                        for b, nt in pe_blocks:
                            c0 = p * UP + PAD + q + NFREE * nt
                            rhs = xts[b][:, c0 : c0 + NFREE]
                            nc.tensor.matmul(
                                pss[b, nt][:], diags[s * W + k][:], rhs,
                                start=(k == 0), stop=(k == W - 1),
                            )
                    for b, nt in pe_blocks:
                        dst = ots[b][:, s * U + NFREE * nt : s * U + NFREE * (nt + 1)]
                        nc.scalar.activation(
                            dst, pss[b, nt][:], mybir.ActivationFunctionType.Identity,
                            bias=bt[:, g * 3 + s : g * 3 + s + 1], scale=1.0,
                        )
                    for b in range(B_LOC):
                        for nt in range(NT):
                            role = block_role(b, s, nt)
                            if role == "pe":
                                continue
                            tmps = []
                            for j in range(W):
                                tv = dv.tile([128, NFREE], f16, tag=f"dv{role}{j}")
                                tmps.append(tv)
                            for k in range(W):
                                o = s + k - (W - 1)
                                p, q = o % 3, o // 3
                                c0 = p * UP + PAD + q + NFREE * nt
                                xsl = xts[b][:, c0 : c0 + NFREE]
                                col = (g * 3 + s) * W + k
                                bias_ap = bt[:, g * 3 + s : g * 3 + s + 1]
                                if role == "coop":
                                    nc.scalar.activation(
                                        tmps[k][:], xsl,
                                        mybir.ActivationFunctionType.Identity,
                                        bias=bias_ap if k == 0 else 0.0,
                                        scale=wt[:, col : col + 1],
                                    )
                                elif k == 0:
                                    nc.vector.tensor_scalar(
                                        tmps[0][:], xsl, wt[:, col : col + 1],
                                        bias_ap,
                                        op0=mybir.AluOpType.mult,
                                        op1=mybir.AluOpType.add,
                                    )
                                else:
                                    nc.vector.tensor_scalar_mul(
                                        tmps[k][:], xsl, wt[:, col : col + 1]
                                    )
                            for a_, b_ in ((0, 1), (2, 3), (4, 5), (6, 7), (0, 2)):
                                nc.vector.tensor_add(tmps[a_][:], tmps[a_][:], tmps[b_][:])
                            nc.vector.tensor_add(tmps[4][:], tmps[4][:], tmps[6][:])
                            dst = ots[b][:, s * U + NFREE * nt : s * U + NFREE * (nt + 1)]
                            nc.vector.tensor_add(dst, tmps[0][:], tmps[4][:])
                for b in range(B_LOC):
                    nc.sync.dma_start(y_d[b, g], ots[b][:])

    nc.compile()
    if not os.environ.get("KERNEL_NO_LDW_DEDUP"):
        n = _dedupe_ldweights(nc)
        if os.environ.get("KERNEL_VERBOSE"):
            print(f"deduped {n} ldweights")
    return nc


def _get_nc():
    if "nc" not in _cache:
        _cache["nc"] = _build_nc()
    return _cache["nc"]


def _install_ntff_hook():
    """antenv.axon_hooks is not shipped in this container; shim it so
    bass_utils can find the NTFF profile hook (trace=True path)."""
    import sys, types
    if "antenv.axon_hooks" in sys.modules:
        return
    mod = types.ModuleType("antenv.axon_hooks")
    mod._hook = None
    mod.set_axon_ntff_profile_hook = lambda h: setattr(mod, "_hook", h)
    mod.get_axon_ntff_profile_hook = lambda: mod._hook
    sys.modules["antenv.axon_hooks"] = mod
    try:
        from trn_agent_boot.trn_boot import _ntff_profile_via_ctypes
        mod._hook = _ntff_profile_via_ctypes("/opt/axon/libaxon_pjrt.so")
    except Exception:
        mod._hook = None


def kernel(x, rtg_w, rtg_b, obs_w, obs_b, act_w, act_b):
    from concourse import bass_utils

    x = np.asarray(x, dtype=np.float32)
    w_sets = [np.asarray(a, dtype=np.float32) for a in (rtg_w, obs_w, act_w)]
    b_sets = [np.asarray(a, dtype=np.float32) for a in (rtg_b, obs_b, act_b)]

    # weights laid out [128 c_local, (g*3+s)*8+k] as f32 values (the on-chip
    # diag build multiplies an fp16 identity by this per-partition f32 scalar)
    w_all = np.zeros((128, G * 3 * W), dtype=np.float32)
    b_all = np.zeros((128, G * 3), dtype=np.float32)
    for g in range(G):
        for s in range(3):
            w_all[:, (g * 3 + s) * W : (g * 3 + s + 1) * W] = w_sets[s][g * 128 : (g + 1) * 128]
            b_all[:, g * 3 + s] = b_sets[s][g * 128 : (g + 1) * 128]
    wid = np.eye(128, dtype=np.float16)

    in_maps = []
    for c in range(NCORES):
        xc = x[c * B_LOC : (c + 1) * B_LOC]                      # [2, T, H]
        x_t = xc.transpose(0, 2, 1).reshape(B_LOC, G, 128, U, 3)
        xph = np.zeros((B_LOC, G, 128, 3, UP), dtype=np.float16)
        xph[..., PAD:] = x_t.transpose(0, 1, 2, 4, 3)            # [b,g,c,p,u]
        in_maps.append({"x": xph.reshape(B_LOC, G, 128, 3 * UP),
                        "wid": wid, "w": w_all, "b": b_all})

    nc = _get_nc()
    trace = bool(int(os.environ.get("KERNEL_TRACE", "0")))
    if trace:
        _install_ntff_hook()
    res = bass_utils.run_bass_kernel_spmd(
        nc, in_maps, core_ids=list(range(NCORES)), trace=trace,
    )
    _cache["last_result"] = res

    out = np.empty((B, T, H), dtype=np.float32)
    for c in range(NCORES):
        y = res.results[c]["y"].astype(np.float32)               # [b,g,c,3*U]
        y = y.reshape(B_LOC, H, 3, U).transpose(0, 1, 3, 2)      # [b,H,u,p]
        y = y.reshape(B_LOC, H, T)
        out[c * B_LOC : (c + 1) * B_LOC] = y.transpose(0, 2, 1)
    return out
